# revision 3
# baseline (speedup 1.0000x reference)
"""Multi-head attention (B=2, S=2048, H=1024, 16 heads x 64) on 8 TRN2 cores.

Sharding: data-parallel over batch (cores 0-3 -> b=0, cores 4-7 -> b=1),
tensor-parallel over heads (4 heads / core, i.e. a 256-column slice of
Wq/Wk/Wv).  Each core computes its heads' full attention; the host
assembles the [B, S, 1024] output from the per-core [S, 256] slices.

Per-core kernel layout (all on-chip, no DRAM spill of scores):
  - QT/KT stored as [head_dim(c), seq] so stage A computes S^T tiles
    [j, i] directly; no max-subtraction is needed (scores are O(1) and
    masked entries underflow to exactly 0 after exp).
  - mask applied multiplicatively after exp:  exp(s/8 - 10000*m)
    == exp(s/8) * (1-m)  exactly in fp32 for m in {0,1}.
  - V is augmented with a ones column, so stage B's matmul also yields
    the softmax denominator for free; a tiny PE transpose at the end
    moves [65, i] tiles to [i, 65] where the divide is per-partition.
  - projections + stage A run in fp32r (full PE rate, ~1e-4 rel err),
    probs/V in bf16 (stage B), everything accumulated in fp32.
"""

import sys

if "/opt/trn_rl_repo" not in sys.path:
    sys.path.insert(0, "/opt/trn_rl_repo")

from contextlib import ExitStack

import ml_dtypes
import numpy as np

import concourse.bass as bass
import concourse.tile as tile
from concourse import bacc, mybir
from concourse.bass_utils import run_bass_kernel_spmd
from concourse.masks import make_identity

F32 = mybir.dt.float32
F32R = mybir.dt.float32r
BF16 = mybir.dt.bfloat16

B, S, H = 2, 2048, 1024
NH, HD = 16, 64
NCORES = 8
CORES_PER_B = NCORES // B          # 4
LHEADS = NH // CORES_PER_B         # 4 heads per core
COLS = LHEADS * HD                 # 256 projection columns per core
HC = H // 128                      # 8 contraction chunks
SBLK = 512                         # seq block (phase 1 moving dim / i-block)
NSB = S // SBLK                    # 4
JT = S // 128                      # 16 j tiles
JG = JT // 2                       # 8 groups of 2 j-tiles (ACT FD=1024)


def build_program():
    nc = bacc.Bacc("TRN2", target_bir_lowering=False, debug=False)

    xT = nc.dram_tensor("xT", [H, S], F32R, kind="ExternalInput").ap()
    maskp = nc.dram_tensor("maskp", [S, S], BF16, kind="ExternalInput").ap()
    wq = nc.dram_tensor("wq", [H, COLS], F32R, kind="ExternalInput").ap()
    wk = nc.dram_tensor("wk", [H, COLS], F32R, kind="ExternalInput").ap()
    wv = nc.dram_tensor("wv", [H, COLS], F32R, kind="ExternalInput").ap()
    bq = nc.dram_tensor("bq", [COLS, 1], F32, kind="ExternalInput").ap()
    bk = nc.dram_tensor("bk", [COLS, 1], F32, kind="ExternalInput").ap()
    bv = nc.dram_tensor("bv", [1, COLS], F32R, kind="ExternalInput").ap()
    ones_d = nc.dram_tensor("ones_d", [1, 128], F32R, kind="ExternalInput").ap()
    out = nc.dram_tensor("out", [S, COLS], F32, kind="ExternalOutput").ap()

    with tile.TileContext(nc) as tc:
        with ExitStack() as ctx:
            # ---- persistent SBUF ----
            persist = ctx.enter_context(tc.tile_pool(name="persist", bufs=1))
            ident = persist.tile([128, 128], F32)
            make_identity(nc, ident[:])
            ones1 = persist.tile([1, 128], F32R)
            nc.sync.dma_start(ones1[:], ones_d[:])
            bq_sb = persist.tile([128, 2], F32)
            bk_sb = persist.tile([128, 2], F32)
            bv_sb = persist.tile([1, COLS], F32R)
            for hp in range(2):
                nc.sync.dma_start(bq_sb[:, hp : hp + 1], bq[hp * 128 : hp * 128 + 128, :])
                nc.sync.dma_start(bk_sb[:, hp : hp + 1], bk[hp * 128 : hp * 128 + 128, :])
            nc.sync.dma_start(bv_sb[:], bv[:])

            # QT/KT: [pair][c within pair (d of head 2p on 0:64, head 2p+1 on
            # 64:128), seq] ; Vp: [s within tile, s-tile, head, 64 vals + one]
            QT = [persist.tile([128, S], F32R, name=f"QT{p}") for p in range(2)]
            KT = [persist.tile([128, S], F32R, name=f"KT{p}") for p in range(2)]
            Vp = persist.tile([128, JT, LHEADS, 66], BF16)
            nc.gpsimd.memset(Vp[:, :, :, 64:65], 1.0)

            # ---- phase 1: projections ----
            with ExitStack() as p1:
                wpool = p1.enter_context(tc.tile_pool(name="wpool", bufs=1))
                xpool = p1.enter_context(tc.tile_pool(name="xpool", bufs=2))
                psqk = p1.enter_context(tc.tile_pool(name="psqk", bufs=2, space="PSUM"))
                psv = p1.enter_context(tc.tile_pool(name="psv", bufs=2, space="PSUM"))

                wq_sb = wpool.tile([128, HC, COLS], F32R)
                wk_sb = wpool.tile([128, HC, COLS], F32R)
                wv_sb = wpool.tile([128, HC, COLS], F32R)
                nc.sync.dma_start(wq_sb[:], wq.rearrange("(c p) n -> p c n", p=128))
                nc.sync.dma_start(wk_sb[:], wk.rearrange("(c p) n -> p c n", p=128))
                nc.sync.dma_start(wv_sb[:], wv.rearrange("(c p) n -> p c n", p=128))

                for sb in range(NSB):
                    xt = xpool.tile([128, HC, SBLK], F32R, name="xt")
                    nc.sync.dma_start(
                        xt[:],
                        xT[:, sb * SBLK : (sb + 1) * SBLK].rearrange(
                            "(c p) s -> p c s", p=128
                        ),
                    )
                    sl = slice(sb * SBLK, (sb + 1) * SBLK)
                    for hp in range(2):
                        cs = slice(hp * 128, hp * 128 + 128)
                        pq = psqk.tile([128, SBLK], F32, name="pq", tag="pq")
                        for hc in range(HC):
                            nc.tensor.matmul(
                                pq[:],
                                wq_sb[:, hc, cs],
                                xt[:, hc, :],
                                start=(hc == 0),
                                stop=(hc == HC - 1),
                            )
                        nc.vector.tensor_scalar_add(
                            QT[hp][:, sl], pq[:], bq_sb[:, hp : hp + 1]
                        )
                        pk = psqk.tile([128, SBLK], F32, name="pk", tag="pk")
                        for hc in range(HC):
                            nc.tensor.matmul(
                                pk[:],
                                wk_sb[:, hc, cs],
                                xt[:, hc, :],
                                start=(hc == 0),
                                stop=(hc == HC - 1),
                            )
                        nc.vector.tensor_scalar_add(
                            KT[hp][:, sl], pk[:], bk_sb[:, hp : hp + 1]
                        )
                    for st4 in range(4):
                        st = sb * 4 + st4
                        pv = psv.tile([128, COLS], F32, name="pv", tag="pv")
                        for hc in range(HC):
                            nc.tensor.matmul(
                                pv[:],
                                xt[:, hc, st4 * 128 : st4 * 128 + 128],
                                wv_sb[:, hc, :],
                                start=(hc == 0),
                                stop=False,
                            )
                        nc.tensor.matmul(
                            pv[:], ones1[:], bv_sb[:], start=False, stop=True
                        )
                        nc.vector.tensor_copy(
                            Vp[:, st, :, 0:64],
                            pv.rearrange("p (h d) -> p h d", h=LHEADS),
                        )

            # ---- phase 2: attention ----
            with ExitStack() as p2:
                mpool = p2.enter_context(tc.tile_pool(name="mpool", bufs=2))
                ppool = p2.enter_context(tc.tile_pool(name="ppool", bufs=1))
                opool = p2.enter_context(tc.tile_pool(name="opool", bufs=2))
                upool = p2.enter_context(tc.tile_pool(name="upool", bufs=2))
                rpool = p2.enter_context(tc.tile_pool(name="rpool", bufs=2))
                pssc = p2.enter_context(tc.tile_pool(name="pssc", bufs=1, space="PSUM"))
                psob = p2.enter_context(tc.tile_pool(name="psob", bufs=1, space="PSUM"))
                pst_p = p2.enter_context(tc.tile_pool(name="pst", bufs=1, space="PSUM"))

                for ib in range(NSB):
                    isl = slice(ib * SBLK, (ib + 1) * SBLK)
                    mt = mpool.tile([128, JT, SBLK], BF16, name="mt")
                    nc.sync.dma_start(
                        mt[:], maskp[:, isl].rearrange("(t p) i -> p t i", p=128)
                    )
                    outt = opool.tile([128, 4, COLS], F32, name="outt")
                    for hp in range(2):
                        P2h = [
                            ppool.tile([128, JT, SBLK], BF16, name=f"P{hl}", tag=f"P{hl}")
                            for hl in range(2)
                        ]
                        for jg in range(JG):
                            for hl in range(2):
                                ps = pssc.tile(
                                    [128, 2, SBLK], F32, name=f"sc{hl}", tag=f"sc{hl}"
                                )
                                rows = slice(hl * 64, hl * 64 + 64)
                                for jj in range(2):
                                    jt_ = jg * 2 + jj
                                    nc.tensor.matmul(
                                        ps[:, jj, :],
                                        KT[hp][rows, jt_ * 128 : jt_ * 128 + 128],
                                        QT[hp][rows, isl],
                                        start=True,
                                        stop=True,
                                    )
                                gsl = slice(jg * 2, jg * 2 + 2)
                                nc.scalar.activation(
                                    P2h[hl][:, gsl, :],
                                    ps[:],
                                    mybir.ActivationFunctionType.Exp,
                                    scale=0.125,
                                )
                                nc.vector.tensor_mul(
                                    P2h[hl][:, gsl, :], P2h[hl][:, gsl, :], mt[:, gsl, :]
                                )
                        for hl in range(2):
                            h = hp * 2 + hl
                            po = psob.tile([65, SBLK], F32, name=f"po{hl}", tag=f"po{hl}")
                            for jt_ in range(JT):
                                nc.tensor.matmul(
                                    po[:],
                                    Vp[:, jt_, h, 0:65],
                                    P2h[hl][:, jt_, :],
                                    start=(jt_ == 0),
                                    stop=(jt_ == JT - 1),
                                )
                            u = upool.tile([65, SBLK], F32, name="u")
                            nc.vector.tensor_copy(u[:], po[:])
                            pt = pst_p.tile([128, 4, 65], F32, name="pt")
                            for c in range(4):
                                nc.tensor.transpose(
                                    pt[:, c, :],
                                    u[:, c * 128 : (c + 1) * 128],
                                    ident[0:65, 0:65],
                                )
                            rec = rpool.tile([128, 4], F32, name="rec")
                            nc.vector.reciprocal(rec[:], pt[:, :, 64])
                            for c in range(4):
                                nc.vector.tensor_scalar_mul(
                                    outt[:, c, h * 64 : h * 64 + 64],
                                    pt[:, c, 0:64],
                                    rec[:, c : c + 1],
                                )
                    nc.sync.dma_start(
                        out[isl, :].rearrange("(c p) n -> p c n", p=128), outt[:]
                    )

    nc.compile()
    return nc


_NC_CACHE = []


def get_nc():
    if not _NC_CACHE:
        _NC_CACHE.append(build_program())
    return _NC_CACHE[0]


def make_in_maps(x, attn_mask, Wq, bq, Wk, bk, Wv, bv):
    x = np.asarray(x, dtype=np.float32)
    attn_mask = np.asarray(attn_mask)
    Wq, Wk, Wv = (np.asarray(w, dtype=np.float32) for w in (Wq, Wk, Wv))
    bq, bk, bv = (np.asarray(b_, dtype=np.float32) for b_ in (bq, bk, bv))

    in_maps = []
    for core in range(NCORES):
        b = core // CORES_PER_B
        hg = core % CORES_PER_B
        cs = slice(hg * COLS, (hg + 1) * COLS)
        mp = (1 - attn_mask[b].T).astype(ml_dtypes.bfloat16)
        in_maps.append(
            {
                "xT": np.ascontiguousarray(x[b].T),
                "maskp": np.ascontiguousarray(mp),
                "wq": np.ascontiguousarray(Wq[:, cs]),
                "wk": np.ascontiguousarray(Wk[:, cs]),
                "wv": np.ascontiguousarray(Wv[:, cs]),
                "bq": np.ascontiguousarray(bq[cs, None]),
                "bk": np.ascontiguousarray(bk[cs, None]),
                "bv": np.ascontiguousarray(bv[None, cs]),
                "ones_d": np.ones((1, 128), np.float32),
            }
        )
    return in_maps


def assemble(results):
    out = np.empty((B, S, H), np.float32)
    for core in range(NCORES):
        b = core // CORES_PER_B
        hg = core % CORES_PER_B
        out[b, :, hg * COLS : (hg + 1) * COLS] = results[core]["out"]
    return out


def kernel(x, attn_mask, Wq, bq, Wk, bk, Wv, bv):
    nc = get_nc()
    in_maps = make_in_maps(x, attn_mask, Wq, bq, Wk, bk, Wv, bv)
    res = run_bass_kernel_spmd(nc, in_maps, list(range(NCORES)))
    return assemble(res.results)
